# revision 1
# baseline (speedup 1.0000x reference)
"""CrystalTransformer (TransformerConv x3 + segment-mean pool) on 8 trn2 cores.

Host: sort edges by dst, shard nodes into 8 contiguous 2560-node ranges
(128-aligned, zero-padded to 20480), pad per-dst-block edge lists to a uniform
tile count so all 8 cores run one SPMD program.

Device per core/layer: per 128-edge tile, gather h[src] + q[dst] (indirect DMA),
ke = [h_src | ea]@W2k (edge-embed folded into weights), alpha = rowdot(q_dst, ke),
ex = exp(alpha/8)*mask, scatter Z_h = S^T @ ([h_src|ea] * ex_h) via one-hot
matmul into per-dst-block PSUM (denominator = ea's ones column). Per block:
Z_h/den, project through Wv2_h (per-head), add skip, relu. AllGather h between
layers; pooling via one-hot matmul on batch ids; final tiny matmul on host.
"""
import json
import numpy as np

P = 128
N, E, G = 20000, 320000, 256
DA, DE, D, H, L = 92, 50, 64, 4, 3
NCORES = 8
NLOC = 2560            # node slots per core (20 blocks of 128)
NB = NLOC // P         # 20 dst blocks per core
NPAD = NLOC * NCORES   # 20480
XW = D + DE + 1        # 115 = [h_src(64) | ea(50) | 1]


# ---------------------------------------------------------------- BIR patch --
def _install_birpatch():
    """This container's walrus rejects >1 sem wait per instruction; hoist
    extras onto injected preceding Drains (same engine => same order)."""
    import concourse.bass2jax as b2j
    if getattr(b2j, "_birpatch_installed", False):
        return
    orig = b2j.compile_bir_kernel

    def patch(bir_bytes):
        d = json.loads(bir_bytes)
        for fn in d.get("functions", []):
            for blk in fn.get("blocks", []):
                out = []
                for ins in blk.get("instructions", []):
                    si = ins.get("sync_info") or {}
                    waits = si.get("on_wait") or []
                    if len(waits) > 1:
                        for k, w in enumerate(waits[:-1]):
                            out.append({
                                "debug": ins.get("debug", 0),
                                "engine": ins["engine"], "ins": [], "outs": [],
                                "name": f'{ins["name"]}-w{k}', "opcode": "Drain",
                                "sync_info": {"on_update": [], "on_wait": [w]},
                            })
                        si["on_wait"] = waits[-1:]
                    out.append(ins)
                blk["instructions"] = out
        return json.dumps(d).encode()

    def wrapper(bir_str, *a, **kw):
        try:
            bir_str = patch(bir_str)
        except Exception as e:  # pragma: no cover
            print("[birpatch] failed:", e)
        return orig(bir_str, *a, **kw)

    b2j.compile_bir_kernel = wrapper
    b2j._birpatch_installed = True


# ------------------------------------------------------------------- device --
def _build_nc(tpb):
    import concourse.bass as bass
    import concourse.mybir as mybir
    import concourse.tile as tile
    from concourse.masks import make_identity

    f32, i32 = mybir.dt.float32, mybir.dt.int32
    Alu, Act = mybir.AluOpType, mybir.ActivationFunctionType
    EB = NB * tpb * P  # padded edges per core

    nc = bass.Bass("TRN2", target_bir_lowering=False, debug=False,
                   num_devices=NCORES)
    di = lambda nm, sh, dt=f32: nc.dram_tensor(nm, sh, dt, kind="ExternalInput")
    x_in = di("x_shard", [NLOC, DA])
    ea_in = di("ea_pad", [EB, DE + 1])
    eaT_in = di("eaT_pad", [DE + 1, EB])
    idx_in = di("idx_i32", [EB, 2], i32)        # [src_global, dst_local]
    meta_in = di("meta_f32", [EB, 2])           # [dst_rel(0..127), mask]
    brel_in = di("batch_rel", [NLOC, 1])
    watom_in = di("w_atom_aug", [DA + 1, D])
    w2k_in = di("w2k", [L, XW, H * D])
    wv2_in = di("wv2", [L, XW, H * D])
    wqs_in = di("wqs", [L, D + 1, H * D + D])
    out_pool = nc.dram_tensor("out_pool", [P, D + 1], f32, kind="ExternalOutput")

    h_mine = nc.dram_tensor("h_mine", [NLOC, D], f32)
    h_full = [nc.dram_tensor(f"h_full_{l}", [NPAD, D], f32, addr_space="Shared")
              for l in range(L)]
    q_dram = [nc.dram_tensor(f"q_dram_{l}", [NLOC, H * D], f32) for l in range(L)]

    with tile.TileContext(nc, num_cores=NCORES) as tc:
        import contextlib
        with contextlib.ExitStack() as st:
            cp = st.enter_context(tc.tile_pool(name="const", bufs=1))
            io = st.enter_context(tc.tile_pool(name="io", bufs=3))
            xp = st.enter_context(tc.tile_pool(name="xt", bufs=3))
            vp = st.enter_context(tc.tile_pool(name="dve", bufs=3))
            bp = st.enter_context(tc.tile_pool(name="blk", bufs=2))
            ps_t = st.enter_context(tc.tile_pool(name="ps_t", bufs=2, space="PSUM"))
            ps_k = st.enter_context(tc.tile_pool(name="ps_k", bufs=2, space="PSUM"))
            ps_z = st.enter_context(tc.tile_pool(name="ps_z", bufs=2, space="PSUM"))
            ps_b = st.enter_context(tc.tile_pool(name="ps_b", bufs=1, space="PSUM"))

            ident = cp.tile([P, P], f32)
            make_identity(nc, ident[:])
            iota_i = cp.tile([P, P], i32)
            nc.gpsimd.iota(iota_i[:], pattern=[[1, P]], base=0, channel_multiplier=0)
            iota_f = cp.tile([P, P], f32)
            nc.vector.tensor_copy(iota_f[:], iota_i[:])
            ones_col = cp.tile([P, 1], f32)
            nc.vector.memset(ones_col[:], 1.0)
            h_loc = cp.tile([P, NB * D], f32)
            skip_loc = cp.tile([P, NB * D], f32)
            watom_sb = cp.tile([DA + 1, D], f32)
            nc.sync.dma_start(out=watom_sb[:], in_=watom_in[:])

            # ---- embed: h0 = relu-free x@W_atom+b (reference has no relu here)
            for b in range(NB):
                xb = io.tile([P, DA], f32)
                nc.sync.dma_start(out=xb[:], in_=x_in[b * P:(b + 1) * P, :])
                xT_ps = ps_t.tile([DA, P], f32, tag="tr")
                nc.tensor.transpose(out=xT_ps[:], in_=xb[:], identity=ident[:])
                xT = xp.tile([DA + 1, P], f32, tag="xt")
                nc.vector.memset(xT[:], 1.0)
                nc.vector.tensor_copy(xT[:DA, :], xT_ps[:])
                hb_ps = ps_b.tile([P, D], f32, tag="blk")
                nc.tensor.matmul(hb_ps[:], lhsT=xT[:], rhs=watom_sb[:],
                                 start=True, stop=True)
                nc.vector.tensor_copy(h_loc[:, b * D:(b + 1) * D], hb_ps[:])
                nc.sync.dma_start(out=h_mine[b * P:(b + 1) * P, :],
                                  in_=h_loc[:, b * D:(b + 1) * D])
            tc.strict_bb_all_engine_barrier()
            nc.gpsimd.collective_compute(
                "AllGather", Alu.bypass,
                replica_groups=[list(range(NCORES))],
                ins=[h_mine.ap().opt()], outs=[h_full[0].ap().opt()])
            tc.strict_bb_all_engine_barrier()

            for l in range(L):
                w2k_sb = cp.tile([XW, H * D], f32, tag="w2k")
                nc.sync.dma_start(out=w2k_sb[:], in_=w2k_in[l])
                wv2_sb = cp.tile([XW, H * D], f32, tag="wv2")
                nc.sync.dma_start(out=wv2_sb[:], in_=wv2_in[l])
                wqs_sb = cp.tile([D + 1, H * D + D], f32, tag="wqs")
                nc.sync.dma_start(out=wqs_sb[:], in_=wqs_in[l])

                # ---- B1: q & skip per block
                for b in range(NB):
                    hT_ps = ps_t.tile([D, P], f32, tag="tr")
                    nc.tensor.transpose(out=hT_ps[:], in_=h_loc[:, b * D:(b + 1) * D],
                                        identity=ident[:])
                    hT = xp.tile([D + 1, P], f32, tag="xt")
                    nc.vector.memset(hT[:], 1.0)
                    nc.vector.tensor_copy(hT[:D, :], hT_ps[:])
                    qs_ps = ps_b.tile([P, H * D + D], f32, tag="blk")
                    nc.tensor.matmul(qs_ps[:], lhsT=hT[:], rhs=wqs_sb[:],
                                     start=True, stop=True)
                    qsb = vp.tile([P, H * D], f32, tag="qsb")
                    nc.vector.tensor_copy(qsb[:], qs_ps[:, :H * D])
                    nc.vector.tensor_copy(skip_loc[:, b * D:(b + 1) * D],
                                          qs_ps[:, H * D:])
                    nc.sync.dma_start(out=q_dram[l][b * P:(b + 1) * P, :], in_=qsb[:])
                tc.strict_bb_all_engine_barrier()

                # ---- B2: edge tiles
                for b in range(NB):
                    z_ps = ps_z.tile([P, H * XW], f32, tag="z")
                    for t in range(tpb):
                        e0 = (b * tpb + t) * P
                        idx = io.tile([P, 2], i32, tag="idx")
                        nc.sync.dma_start(out=idx[:], in_=idx_in[e0:e0 + P, :])
                        met = io.tile([P, 2], f32, tag="met")
                        nc.sync.dma_start(out=met[:], in_=meta_in[e0:e0 + P, :])
                        X = xp.tile([P, XW - 1], f32, tag="X")
                        nc.gpsimd.indirect_dma_start(
                            out=X[:, :D], out_offset=None,
                            in_=h_full[l][:],
                            in_offset=bass.IndirectOffsetOnAxis(ap=idx[:, 0:1], axis=0))
                        nc.sync.dma_start(out=X[:, D:], in_=ea_in[e0:e0 + P, :DE])
                        qd = vp.tile([P, H * D], f32, tag="qd")
                        nc.gpsimd.indirect_dma_start(
                            out=qd[:], out_offset=None,
                            in_=q_dram[l][:],
                            in_offset=bass.IndirectOffsetOnAxis(ap=idx[:, 1:2], axis=0))
                        hsT_ps = ps_t.tile([D, P], f32, tag="tr")
                        nc.tensor.transpose(out=hsT_ps[:], in_=X[:, :D],
                                            identity=ident[:])
                        XT = xp.tile([XW, P], f32, tag="XT")
                        nc.scalar.copy(XT[:D, :], hsT_ps[:])
                        nc.sync.dma_start(out=XT[D:, :], in_=eaT_in[:, e0:e0 + P])
                        ke_ps = ps_k.tile([P, H * D], f32, tag="ke")
                        nc.tensor.matmul(ke_ps[:], lhsT=XT[:], rhs=w2k_sb[:],
                                         start=True, stop=True)
                        prod = vp.tile([P, H * D], f32, tag="prod")
                        nc.vector.tensor_tensor(out=prod[:], in0=ke_ps[:],
                                                in1=qd[:], op=Alu.mult)
                        alpha = vp.tile([P, H], f32, tag="alpha")
                        nc.vector.tensor_reduce(
                            out=alpha[:],
                            in_=prod[:].rearrange("p (h d) -> p h d", d=D),
                            axis=mybir.AxisListType.X, op=Alu.add)
                        ex = vp.tile([P, H], f32, tag="ex")
                        nc.scalar.activation(ex[:], alpha[:], Act.Exp,
                                             scale=float(1.0 / np.sqrt(D)))
                        exm = vp.tile([P, H], f32, tag="exm")
                        nc.vector.tensor_scalar_mul(out=exm[:], in0=ex[:],
                                                    scalar1=met[:, 1:2])
                        Xex = vp.tile([P, H * XW], f32, tag="Xex")
                        for h in range(H):
                            nc.vector.tensor_scalar_mul(
                                out=Xex[:, h * XW:h * XW + XW - 1], in0=X[:],
                                scalar1=exm[:, h:h + 1])
                            nc.vector.tensor_copy(
                                out=Xex[:, h * XW + XW - 1:(h + 1) * XW],
                                in_=exm[:, h:h + 1])
                        S = vp.tile([P, P], f32, tag="S")
                        nc.gpsimd.tensor_scalar(out=S[:], in0=iota_f[:],
                                                scalar1=met[:, 0:1], scalar2=None,
                                                op0=Alu.is_equal)
                        nc.tensor.matmul(z_ps[:], lhsT=S[:], rhs=Xex[:],
                                         start=(t == 0), stop=(t == tpb - 1))

                    # ---- B3: combine per block
                    den = vp.tile([P, H], f32, tag="den")
                    nc.vector.tensor_scalar_max(
                        out=den[:],
                        in0=z_ps[:].rearrange("p (h c) -> p h c", c=XW)[:, :, XW - 1:XW],
                        scalar1=1e-30)
                    rden = vp.tile([P, H], f32, tag="rden")
                    nc.vector.reciprocal(rden[:], den[:])
                    Zn = vp.tile([P, H * XW], f32, tag="Zn")
                    for h in range(H):
                        nc.vector.tensor_scalar_mul(
                            out=Zn[:, h * XW:(h + 1) * XW],
                            in0=z_ps[:, h * XW:(h + 1) * XW],
                            scalar1=rden[:, h:h + 1])
                    agg_ps = ps_b.tile([P, D], f32, tag="blk")
                    for h in range(H):
                        zT_ps = ps_t.tile([XW, P], f32, tag="tr")
                        nc.tensor.transpose(out=zT_ps[:],
                                            in_=Zn[:, h * XW:(h + 1) * XW],
                                            identity=ident[:])
                        zT = xp.tile([XW, P], f32, tag="zT")
                        nc.scalar.copy(zT[:], zT_ps[:])
                        nc.tensor.matmul(agg_ps[:], lhsT=zT[:],
                                         rhs=wv2_sb[:, h * D:(h + 1) * D],
                                         start=(h == 0), stop=(h == H - 1))
                    tmp = vp.tile([P, D], f32, tag="tmp")
                    nc.vector.tensor_tensor(out=tmp[:], in0=agg_ps[:],
                                            in1=skip_loc[:, b * D:(b + 1) * D],
                                            op=Alu.add)
                    nc.vector.tensor_scalar_max(
                        out=h_loc[:, b * D:(b + 1) * D], in0=tmp[:], scalar1=0.0)
                    if l < L - 1:
                        nc.sync.dma_start(out=h_mine[b * P:(b + 1) * P, :],
                                          in_=h_loc[:, b * D:(b + 1) * D])
                if l < L - 1:
                    tc.strict_bb_all_engine_barrier()
                    nc.gpsimd.collective_compute(
                        "AllGather", Alu.bypass,
                        replica_groups=[list(range(NCORES))],
                        ins=[h_mine.ap().opt()], outs=[h_full[l + 1].ap().opt()])
                    tc.strict_bb_all_engine_barrier()

            # ---- pooling: one-hot on batch ids
            brel = cp.tile([P, NB], f32)
            nc.sync.dma_start(out=brel[:],
                              in_=brel_in[:].rearrange("(b p) o -> p (b o)", p=P))
            pool_ps = ps_z.tile([P, D], f32, tag="z")
            cnt_ps = ps_b.tile([P, 1], f32, tag="cnt")
            for b in range(NB):
                Sb = vp.tile([P, P], f32, tag="S")
                nc.vector.tensor_scalar(out=Sb[:], in0=iota_f[:],
                                        scalar1=brel[:, b:b + 1], scalar2=None,
                                        op0=Alu.is_equal)
                nc.tensor.matmul(pool_ps[:], lhsT=Sb[:],
                                 rhs=h_loc[:, b * D:(b + 1) * D],
                                 start=(b == 0), stop=(b == NB - 1))
                nc.tensor.matmul(cnt_ps[:], lhsT=Sb[:], rhs=ones_col[:],
                                 start=(b == 0), stop=(b == NB - 1),
                                 skip_group_check=True)
            pool_sb = vp.tile([P, D + 1], f32, tag="pool_sb")
            nc.vector.tensor_copy(pool_sb[:, :D], pool_ps[:])
            nc.vector.tensor_copy(pool_sb[:, D:], cnt_ps[:])
            nc.sync.dma_start(out=out_pool[:], in_=pool_sb[:])
    return nc


# --------------------------------------------------------------------- host --
def kernel(**inputs):
    _install_birpatch()
    from concourse.bass_utils import run_bass_kernel_spmd

    x = np.asarray(inputs["x"], np.float32)
    ei = np.asarray(inputs["edge_index"]).astype(np.int64)
    ea = np.asarray(inputs["edge_attr"], np.float32)
    batch = np.asarray(inputs["batch"]).astype(np.int64)
    Wq = np.asarray(inputs["Wq"], np.float32); bq = np.asarray(inputs["bq"], np.float32)
    Wk = np.asarray(inputs["Wk"], np.float32); bk = np.asarray(inputs["bk"], np.float32)
    Wv = np.asarray(inputs["Wv"], np.float32); bv = np.asarray(inputs["bv"], np.float32)
    We = np.asarray(inputs["We"], np.float32)
    Wskip = np.asarray(inputs["Wskip"], np.float32)
    bskip = np.asarray(inputs["bskip"], np.float32)
    W_atom = np.asarray(inputs["W_atom"], np.float32)
    b_atom = np.asarray(inputs["b_atom"], np.float32)
    W_edge = np.asarray(inputs["W_edge"], np.float32)
    b_edge = np.asarray(inputs["b_edge"], np.float32)
    W_out = np.asarray(inputs["W_out"], np.float32)
    b_out = np.asarray(inputs["b_out"], np.float32)

    src, dst = ei[0], ei[1]
    order = np.argsort(dst, kind="stable")
    src_s, dst_s = src[order], dst[order]
    ea_s = ea[order]

    # per-(core, block) edge ranges; uniform tile count tpb across all
    blk_of = dst_s // P                       # 0..159 (20 blocks x 8 cores)
    nblk = NCORES * NB
    counts = np.bincount(blk_of, minlength=nblk)
    starts = np.zeros(nblk + 1, np.int64)
    np.cumsum(counts, out=starts[1:])
    tpb = int(np.ceil(max(1, counts.max()) / P))
    EB = NB * tpb * P

    # edge-embed fold: W2k rows = [Wk ; W_edge_aug @ We (+bk)], per layer
    Wea = np.concatenate([W_edge, b_edge[None, :]], 0)        # [51, 64]
    w2k = np.zeros((L, XW, H * D), np.float32)
    wv2 = np.zeros((L, H, XW, D), np.float32)
    wqs = np.zeros((L, D + 1, H * D + D), np.float32)
    for l in range(L):
        ew = Wea @ We[l]                                      # [51, 256]
        w2k[l, :D] = Wk[l]
        w2k[l, D:] = ew
        w2k[l, -1] += bk[l]
        for h in range(H):
            wv2[l, h, :D] = Wv[l][:, h * D:(h + 1) * D] / H
            wv2[l, h, D:] = ew[:, h * D:(h + 1) * D] / H
            wv2[l, h, -1] += bv[l][h * D:(h + 1) * D] / H
        wqs[l, :D, :H * D] = Wq[l]
        wqs[l, D, :H * D] = bq[l]
        wqs[l, :D, H * D:] = Wskip[l]
        wqs[l, D, H * D:] = bskip[l]
    watom = np.concatenate([W_atom, b_atom[None, :]], 0)

    in_maps, g0s = [], []
    for c in range(NCORES):
        n0 = c * NLOC
        xs = np.zeros((NLOC, DA), np.float32)
        real = min(NLOC, max(0, N - n0))
        xs[:real] = x[n0:n0 + real]
        eap = np.zeros((EB, DE + 1), np.float32)
        idx = np.zeros((EB, 2), np.int32)
        met = np.zeros((EB, 2), np.float32)
        for b in range(NB):
            gb = c * NB + b
            s, e = starts[gb], starts[gb + 1]
            k = e - s
            o = b * tpb * P
            eap[o:o + k, :DE] = ea_s[s:e]
            eap[o:o + k, DE] = 1.0
            idx[o:o + k, 0] = src_s[s:e]
            idx[o:o + k, 1] = dst_s[s:e] - n0
            met[o:o + k, 0] = dst_s[s:e] - (n0 + b * P)
            met[o:o + k, 1] = 1.0
        brel = np.full((NLOC, 1), -1.0, np.float32)
        g0 = int(batch[min(n0, N - 1)]) if n0 < N else 0
        if real > 0:
            brel[:real, 0] = batch[n0:n0 + real] - g0
        g0s.append(g0)
        in_maps.append({
            "x_shard": xs, "ea_pad": eap,
            "eaT_pad": np.ascontiguousarray(eap.T),
            "idx_i32": idx, "meta_f32": met, "batch_rel": brel,
            "w_atom_aug": watom, "w2k": w2k,
            "wv2": np.ascontiguousarray(np.transpose(wv2, (0, 2, 1, 3))
                                        .reshape(L, XW, H * D)),
            "wqs": wqs,
        })

    nc = _build_nc(tpb)
    res = run_bass_kernel_spmd(nc, in_maps, core_ids=list(range(NCORES)))

    sums = np.zeros((G + P, D), np.float64)
    cnts = np.zeros(G + P, np.float64)
    for c in range(NCORES):
        op = res.results[c]["out_pool"]
        sums[g0s[c]:g0s[c] + P] += op[:, :D]
        cnts[g0s[c]:g0s[c] + P] += op[:, D]
    pooled = sums[:G] / np.maximum(cnts[:G], 1.0)[:, None]
    out = pooled.astype(np.float32) @ W_out + b_out
    return out.squeeze()

